# revision 3
# baseline (speedup 1.0000x reference)
"""Trainium2 Bass kernel v2 for per-class variance-trace (segment reduction).

Computes, for x[N, D] (fp32) and t[N] (int32 class ids in [0, 10)):
    out = mean_c( sum_d unbiased_var(x[t == c, d]) )

Strategy (8-way data parallel over N):
  - x streams in via SWDGE (gpsimd) DMA with an inline fp32->fp16 cast
    (round-to-nearest, verified on HW). Halving the SBUF write side
    raises the achieved stream rate from ~323 GB/s (fp32 HWDGE) to
    ~393 GB/s (measured, all 8 cores active).
  - Per 128-row subtile the vector engine builds a one-hot O[128, 10]
    from t and squares x (fp16 2x mode). The tensor engine accumulates
    both segment sums in ONE matmul per subtile into a [10, 256] PSUM
    tile: rhs = [x | x^2] via a 2-level access pattern.
  - Counts come from np.bincount on the host (t is tiny); the final
    variance/trace arithmetic happens on the host in float64.

  Uncentered sum-of-squares is numerically safe here: means are ~0 so
  the correction term sums^2/count is ~1e-5 of ssq; fp16 RN rounding
  of x gives a variance bias ~1e-7 relative.
"""

import sys

sys.path.insert(0, "/opt/trn_rl_repo")

import numpy as np

NUM_CLASSES = 10
N = 1_000_000
D = 128
P = 128
NCORES = 8
NSHARD = N // NCORES  # 125_000 rows per core

G = 8  # subtiles per group (0.52 MB fp32 read per x DMA)
XBUFS = 12  # fp16 x-tile ring depth (DMA in-flight depth)

_CACHE = {}


def _build(ns, g=G, xbufs=XBUFS):
    """Build + compile the per-core Bass program for a shard of `ns` rows."""
    from concourse import bacc, mybir
    import concourse.tile as tile

    f32 = mybir.dt.float32
    f16 = mybir.dt.float16
    i32 = mybir.dt.int32
    eq = mybir.AluOpType.is_equal
    mult = mybir.AluOpType.mult
    C = NUM_CLASSES

    qmain = ns // P
    tail = ns - qmain * P

    # Group schedule: uniform g-sized groups with a short taper at the end
    # so the final DMA's dependent compute chain is tiny.
    groups = []
    pos = 0
    while qmain - pos > g:
        groups.append((pos, g))
        pos += g
    rem = qmain - pos
    while rem > 0:
        take = (rem + 1) // 2 if rem > 1 else 1
        groups.append((pos, take))
        pos += take
        rem -= take
    assert pos == qmain and sum(gl for _, gl in groups) == qmain

    nc = bacc.Bacc("TRN2", target_bir_lowering=False, debug=False)
    x_d = nc.dram_tensor("x", [ns, D], f32, kind="ExternalInput")
    t_d = nc.dram_tensor("t", [ns], i32, kind="ExternalInput")
    out_d = nc.dram_tensor("out", [C, 2 * D], f32, kind="ExternalOutput")

    # Row mapping: partition p of subtile q holds DRAM row p*qmain + q, so a
    # group of g subtiles is a contiguous g-row (g*D*4 byte) read per partition.
    x_main = x_d.ap()[0 : qmain * P, :].rearrange("(p q) d -> p q d", p=P)
    t_main = t_d.ap()[0 : qmain * P].rearrange("(p q) -> p q", p=P)

    with tile.TileContext(nc) as tc:
        with (
            tc.tile_pool(name="xs", bufs=xbufs) as xpool,
            tc.tile_pool(name="singles", bufs=1) as singles,
            tc.tile_pool(name="psum", bufs=1, space="PSUM") as psum,
        ):
            # The x stream owns the SWDGE (gpsimd) queue: issue the first two
            # group DMAs before anything else lands on that queue so the
            # stream starts immediately. t goes via the idle sync HWDGE queue.
            pre = {}
            for gi in range(min(2, len(groups))):
                i0, gl = groups[gi]
                xs = xpool.tile([P, 2, gl, D], f16, tag="xs")
                nc.gpsimd.dma_start(out=xs[:, 0], in_=x_main[:, i0 : i0 + gl, :])
                pre[gi] = xs

            t_all_i = singles.tile([P, qmain], i32)
            nc.sync.dma_start(out=t_all_i[:], in_=t_main)
            t_all = singles.tile([P, qmain], f16)
            nc.vector.tensor_copy(t_all[:], t_all_i[:])
            iota10_i = singles.tile([P, C], i32)
            nc.gpsimd.iota(iota10_i[:], pattern=[[1, C]], base=0, channel_multiplier=0)
            iota10 = singles.tile([P, C], f16)
            nc.vector.tensor_copy(iota10[:], iota10_i[:])

            # Pre-build ALL one-hots in one DVE op (only needs t, runs during
            # the first group DMAs): ogb_all[p, q, c] = (t[p, q] == c).
            ogb_all = singles.tile([P, qmain, C], f16)
            nc.vector.tensor_tensor(
                out=ogb_all[:],
                in0=t_all[:, :, None].to_broadcast([P, qmain, C]),
                in1=iota10[:, None, :].to_broadcast([P, qmain, C]),
                op=eq,
            )

            p_acc = psum.tile([C, 2 * D], f32)

            # Ragged tail FIRST: `tail` leftover rows go into partitions
            # [0, tail) of one extra subtile (unused partitions zeroed so
            # they add 0), and its matmul OPENS the PSUM accumulation group.
            # Doing this up front keeps the end-of-stream critical chain to
            # just the final 1-subtile taper chunk.
            xt = singles.tile([P, 2, 1, D], f16)
            nc.vector.memset(xt[:], 0.0)
            otb = singles.tile([P, C], f16)
            nc.vector.memset(otb[:], 0.0)
            if tail:
                tt_i = singles.tile([P, 1], i32)
                tt = singles.tile([P, 1], f16)
                nc.gpsimd.dma_start(
                    out=xt[0:tail, 0, 0, :], in_=x_d.ap()[qmain * P : ns, :]
                )
                nc.sync.dma_start(
                    out=tt_i[0:tail, :], in_=t_d.ap()[qmain * P : ns, None]
                )
                nc.vector.tensor_copy(tt[0:tail, :], tt_i[0:tail, :])
                nc.vector.tensor_tensor(
                    out=otb[0:tail, :],
                    in0=tt[0:tail, 0:1].to_broadcast([tail, C]),
                    in1=iota10[0:tail, :],
                    op=eq,
                )
            nc.vector.tensor_tensor(
                out=xt[:, 1, 0, :], in0=xt[:, 0, 0, :], in1=xt[:, 0, 0, :], op=mult
            )
            nc.tensor.matmul(
                out=p_acc[:], lhsT=otb[:], rhs=xt[:, :, 0, :], start=True, stop=False
            )

            n_mm = sum(gl for _, gl in groups)
            mm = 0
            for gi, (i0, gl) in enumerate(groups):
                # [P, 2, gl, D]: half 0 = x (landed fp16 via DMA cast),
                # half 1 = x^2 (DVE). MM rhs for subtile k is [:, :, k, :]
                # -> a 256-wide 2-level AP, one matmul per subtile.
                if gi in pre:
                    xs = pre[gi]
                else:
                    xs = xpool.tile([P, 2, gl, D], f16, tag="xs")
                    nc.gpsimd.dma_start(out=xs[:, 0], in_=x_main[:, i0 : i0 + gl, :])
                nc.vector.tensor_tensor(
                    out=xs[:, 1], in0=xs[:, 0], in1=xs[:, 0], op=mult
                )

                for k in range(gl):
                    mm += 1
                    nc.tensor.matmul(
                        out=p_acc[:],
                        lhsT=ogb_all[:, i0 + k, :],
                        rhs=xs[:, :, k, :],
                        start=False,
                        stop=(mm == n_mm),
                    )

            out_sb = singles.tile([C, 2 * D], f32)
            nc.scalar.copy(out_sb[:], p_acc[:])
            nc.sync.dma_start(out=out_d.ap()[:], in_=out_sb[:])

    nc.compile()
    return nc, "out"


def _get_program(ns, g=G, xbufs=XBUFS):
    key = (ns, g, xbufs)
    if key not in _CACHE:
        _CACHE[key] = _build(ns, g, xbufs)
    return _CACHE[key]


def _finalize(partials, counts):
    """partials: [ncores, C, 2D]; counts: [C] -> final [1] fp32."""
    acc = partials.astype(np.float64).sum(axis=0)
    sums = acc[:, 0:D]
    ssq = acc[:, D : 2 * D]
    cnt = counts.astype(np.float64)
    s2 = ssq.sum(axis=1)
    corr = (sums * sums).sum(axis=1) / cnt
    trace_per_class = (s2 - corr) / (cnt - 1.0)
    result = trace_per_class.sum() / NUM_CLASSES
    return np.asarray([result], dtype=np.float32)


def kernel(x, t):
    from concourse.bass_utils import run_bass_kernel_spmd

    x = np.ascontiguousarray(np.asarray(x, dtype=np.float32))
    t = np.ascontiguousarray(np.asarray(t, dtype=np.int32))
    assert x.shape == (N, D) and t.shape == (N,), (x.shape, t.shape)

    nc, out_name = _get_program(NSHARD)
    in_maps = [
        {
            "x": x[k * NSHARD : (k + 1) * NSHARD],
            "t": t[k * NSHARD : (k + 1) * NSHARD],
        }
        for k in range(NCORES)
    ]
    res = run_bass_kernel_spmd(nc, in_maps, core_ids=list(range(NCORES)))
    partials = np.stack([res.results[k][out_name] for k in range(NCORES)])
    counts = np.bincount(t, minlength=NUM_CLASSES)
    return _finalize(partials, counts)


# revision 4
# speedup vs baseline: 1.0772x; 1.0772x over previous
"""Trainium2 Bass kernel v2 for per-class variance-trace (segment reduction).

Computes, for x[N, D] (fp32) and t[N] (int32 class ids in [0, 10)):
    out = mean_c( sum_d unbiased_var(x[t == c, d]) )

Strategy (8-way data parallel over N):
  - x streams in via SWDGE (gpsimd) DMA with an inline fp32->fp16 cast
    (round-to-nearest, verified on HW). Halving the SBUF write side
    raises the achieved stream rate from ~323 GB/s (fp32 HWDGE) to
    ~393 GB/s (measured, all 8 cores active).
  - Per 128-row subtile the vector engine builds a one-hot O[128, 10]
    from t and squares x (fp16 2x mode). The tensor engine accumulates
    both segment sums in ONE matmul per subtile into a [10, 256] PSUM
    tile: rhs = [x | x^2] via a 2-level access pattern.
  - Counts come from np.bincount on the host (t is tiny); the final
    variance/trace arithmetic happens on the host in float64.

  Uncentered sum-of-squares is numerically safe here: means are ~0 so
  the correction term sums^2/count is ~1e-5 of ssq; fp16 RN rounding
  of x gives a variance bias ~1e-7 relative.
"""

import sys

sys.path.insert(0, "/opt/trn_rl_repo")

import numpy as np

NUM_CLASSES = 10
N = 1_000_000
D = 128
P = 128
NCORES = 8
NSHARD = N // NCORES  # 125_000 rows per core

G = 8  # subtiles per group (0.52 MB fp32 read per x DMA)
XBUFS = 12  # fp16 x-tile ring depth (DMA in-flight depth)

_CACHE = {}


def _build(ns, g=G, xbufs=XBUFS):
    """Build + compile the per-core Bass program for a shard of `ns` rows."""
    from concourse import bacc, mybir
    import concourse.tile as tile

    f32 = mybir.dt.float32
    f16 = mybir.dt.float16
    i32 = mybir.dt.int32
    eq = mybir.AluOpType.is_equal
    mult = mybir.AluOpType.mult
    C = NUM_CLASSES

    qmain = ns // P
    tail = ns - qmain * P

    # Group schedule: uniform g-sized groups with a short taper at the end
    # so the final DMA's dependent compute chain is tiny.
    groups = []
    pos = 0
    while qmain - pos > g:
        groups.append((pos, g))
        pos += g
    rem = qmain - pos
    while rem > 0:
        take = (rem + 1) // 2 if rem > 1 else 1
        groups.append((pos, take))
        pos += take
        rem -= take
    assert pos == qmain and sum(gl for _, gl in groups) == qmain

    nc = bacc.Bacc("TRN2", target_bir_lowering=False, debug=False)
    x_d = nc.dram_tensor("x", [ns, D], f32, kind="ExternalInput")
    t_d = nc.dram_tensor("t", [ns], i32, kind="ExternalInput")
    out_d = nc.dram_tensor("out", [C, 2 * D], f32, kind="ExternalOutput")

    # Row mapping: partition p of subtile q holds DRAM row p*qmain + q, so a
    # group of g subtiles is a contiguous g-row (g*D*4 byte) read per partition.
    x_main = x_d.ap()[0 : qmain * P, :].rearrange("(p q) d -> p q d", p=P)
    t_main = t_d.ap()[0 : qmain * P].rearrange("(p q) -> p q", p=P)

    with tile.TileContext(nc) as tc:
        with (
            tc.tile_pool(name="xs", bufs=xbufs) as xpool,
            tc.tile_pool(name="singles", bufs=1) as singles,
            tc.tile_pool(name="psum", bufs=1, space="PSUM") as psum,
        ):
            # The x stream owns the SWDGE (gpsimd) queue: issue the first two
            # group DMAs before anything else lands on that queue so the
            # stream starts immediately. t goes via the idle sync HWDGE queue.
            pre = {}
            for gi in range(min(2, len(groups))):
                i0, gl = groups[gi]
                xs = xpool.tile([P, 2, gl, D], f16, tag="xs")
                nc.gpsimd.dma_start(out=xs[:, 0], in_=x_main[:, i0 : i0 + gl, :])
                pre[gi] = xs

            t_all_i = singles.tile([P, qmain], i32)
            nc.sync.dma_start(out=t_all_i[:], in_=t_main)
            t_all = singles.tile([P, qmain], f16)
            nc.vector.tensor_copy(t_all[:], t_all_i[:])
            iota10_i = singles.tile([P, C], i32)
            nc.gpsimd.iota(iota10_i[:], pattern=[[1, C]], base=0, channel_multiplier=0)
            iota10 = singles.tile([P, C], f16)
            nc.vector.tensor_copy(iota10[:], iota10_i[:])

            # Pre-build ALL one-hots in one DVE op (only needs t, runs during
            # the first group DMAs): ogb_all[p, q, c] = (t[p, q] == c).
            ogb_all = singles.tile([P, qmain, C], f16)
            nc.vector.tensor_tensor(
                out=ogb_all[:],
                in0=t_all[:, :, None].to_broadcast([P, qmain, C]),
                in1=iota10[:, None, :].to_broadcast([P, qmain, C]),
                op=eq,
            )

            p_acc = psum.tile([C, 2 * D], f32)

            first = True
            for gi, (i0, gl) in enumerate(groups):
                # [P, 2, gl, D]: half 0 = x (landed fp16 via DMA cast),
                # half 1 = x^2 (DVE). MM rhs for subtile k is [:, :, k, :]
                # -> a 256-wide 2-level AP, one matmul per subtile.
                if gi in pre:
                    xs = pre[gi]
                else:
                    xs = xpool.tile([P, 2, gl, D], f16, tag="xs")
                    nc.gpsimd.dma_start(out=xs[:, 0], in_=x_main[:, i0 : i0 + gl, :])
                nc.vector.tensor_tensor(
                    out=xs[:, 1], in0=xs[:, 0], in1=xs[:, 0], op=mult
                )

                for k in range(gl):
                    nc.tensor.matmul(
                        out=p_acc[:],
                        lhsT=ogb_all[:, i0 + k, :],
                        rhs=xs[:, :, k, :],
                        start=first,
                        stop=False,
                    )
                    first = False

            # Ragged tail: `tail` leftover rows go into partitions [0, tail) of
            # one extra subtile; unused partitions are zeroed so they add 0.
            xt = singles.tile([P, 2, 1, D], f16)
            nc.vector.memset(xt[:], 0.0)
            otb = singles.tile([P, C], f16)
            nc.vector.memset(otb[:], 0.0)
            if tail:
                tt_i = singles.tile([P, 1], i32)
                tt = singles.tile([P, 1], f16)
                nc.gpsimd.dma_start(
                    out=xt[0:tail, 0, 0, :], in_=x_d.ap()[qmain * P : ns, :]
                )
                nc.sync.dma_start(
                    out=tt_i[0:tail, :], in_=t_d.ap()[qmain * P : ns, None]
                )
                nc.vector.tensor_copy(tt[0:tail, :], tt_i[0:tail, :])
                nc.vector.tensor_tensor(
                    out=otb[0:tail, :],
                    in0=tt[0:tail, 0:1].to_broadcast([tail, C]),
                    in1=iota10[0:tail, :],
                    op=eq,
                )
            nc.vector.tensor_tensor(
                out=xt[:, 1, 0, :], in0=xt[:, 0, 0, :], in1=xt[:, 0, 0, :], op=mult
            )
            nc.tensor.matmul(
                out=p_acc[:], lhsT=otb[:], rhs=xt[:, :, 0, :], start=first, stop=True
            )

            out_sb = singles.tile([C, 2 * D], f32)
            nc.scalar.copy(out_sb[:], p_acc[:])
            nc.sync.dma_start(out=out_d.ap()[:], in_=out_sb[:])

    nc.compile()
    return nc, "out"


def _get_program(ns, g=G, xbufs=XBUFS):
    key = (ns, g, xbufs)
    if key not in _CACHE:
        _CACHE[key] = _build(ns, g, xbufs)
    return _CACHE[key]


def _finalize(partials, counts):
    """partials: [ncores, C, 2D]; counts: [C] -> final [1] fp32."""
    acc = partials.astype(np.float64).sum(axis=0)
    sums = acc[:, 0:D]
    ssq = acc[:, D : 2 * D]
    cnt = counts.astype(np.float64)
    s2 = ssq.sum(axis=1)
    corr = (sums * sums).sum(axis=1) / cnt
    trace_per_class = (s2 - corr) / (cnt - 1.0)
    result = trace_per_class.sum() / NUM_CLASSES
    return np.asarray([result], dtype=np.float32)


def kernel(x, t):
    from concourse.bass_utils import run_bass_kernel_spmd

    x = np.ascontiguousarray(np.asarray(x, dtype=np.float32))
    t = np.ascontiguousarray(np.asarray(t, dtype=np.int32))
    assert x.shape == (N, D) and t.shape == (N,), (x.shape, t.shape)

    nc, out_name = _get_program(NSHARD)
    in_maps = [
        {
            "x": x[k * NSHARD : (k + 1) * NSHARD],
            "t": t[k * NSHARD : (k + 1) * NSHARD],
        }
        for k in range(NCORES)
    ]
    res = run_bass_kernel_spmd(nc, in_maps, core_ids=list(range(NCORES)))
    partials = np.stack([res.results[k][out_name] for k in range(NCORES)])
    counts = np.bincount(t, minlength=NUM_CLASSES)
    return _finalize(partials, counts)
